# revision 11
# baseline (speedup 1.0000x reference)
"""Trainium2 Bass kernel for nn_ASTEmbeder (AST code/desc attention-pool + hinge loss).

V2 strategy (data-parallel over batch, 8 cores, fp8 everywhere):
- Samples are slotted so every core runs one SPMD program: code trees sorted
  by size, desc streams sorted by length (independently per stream), slot j =
  sorted ranks [8j, 8j+8). Slot width = max len in slot (64-padded), so short
  descriptions are never loaded at all.
- Host prep: h = tanh(all_node_h) on host; streams quantized to fp8e4m3,
  transposed (H on partitions), one contiguous (128, L) byte stream per core
  -> 10-16KB/partition chunk DMAs.
- No masks: padded rows are zeroed in the stream. A zero row contributes 0 to
  the pooled sum and exactly exp(v.tanh(b)) (=1 for b=0) to z; the host
  subtracts pad_count * exp(v.tanh(b)) from z.
- On-chip per group of n<=512 cols (rows of the original layout):
    t = W.T @ h.T      16 fp8 DoubleRow matmuls (256-wide K pairs) into one
                       4-bank PSUM tile
    u = tanh(t)        1 ACT instr over all 4 banks, fp8 out (b==0 fast path)
    s = v.T @ u        4 fp8 DoubleRow matmuls, v replicated over 128 cols
    e = exp(s)         ACT, fp8 out, z via accum_out
    pooled[mc] += ...  4 DVE scalar_tensor_tensor with f32 accum columns
- Emission is software-pipelined: group g's v/exp/pool instructions are
  emitted after group g+1's W/tanh, so PE never waits on ACT.
- Host: repr = tanh(pooled/z), cosine sims, hinge loss. Softmax
  max-subtraction dropped (logits O(1)); v1b/v2b dropped (shift-invariance).
"""
import contextlib
import os
import numpy as np

B, H, S = 256, 512, 512
NCORES = 8
MARGIN, EPS = 0.05, 1e-8

LAST_RESULTS = None
_PROGRAM_CACHE = {}
_RUN_STATE = {}  # cached jitted executable + device-resident inputs (timing)


def _np_f8():
    import ml_dtypes
    return ml_dtypes.float8_e4m3fn


def _split_rows(n, cap=512):
    """Split n rows into near-equal groups <= cap, multiples of 32 except the
    last."""
    if n <= cap:
        return [n]
    k = (n + cap - 1) // cap
    base = (n // k + 31) // 32 * 32
    out, left = [], n
    for _ in range(k - 1):
        out.append(base)
        left -= base
    out.append(left)
    return out


def _plan(node_num, anc_len, neg_len):
    """Slot assignment + group/chunk plan shared by all cores (SPMD).

    Every stream independently sorts its samples so slot j holds sorted ranks
    [8j, 8j+8) (core k takes rank 8j+k); slot width = 64-padded max length.

    Returns (slot_samples, slot_rows, groups, chunks, total_cols) where
      slot_samples[(stream, j)][k] = sample id for core k
      slot_rows[(stream, j)] = padded row count of the slot
      groups: dicts(stream, j, row0, n, off)
      chunks: dicts(g0, ng, n, off)  -- ng consecutive groups of equal n
    """
    lens = {0: np.asarray(node_num), 1: np.asarray(anc_len),
            2: np.asarray(neg_len)}
    nslots = B // NCORES
    slot_samples, slot_rows = {}, {}
    groups = []
    for stream in (0, 1, 2):
        order = np.argsort(lens[stream], kind="stable")
        for j in range(nslots):
            ranks = [int(r) for r in order[j * NCORES:(j + 1) * NCORES]]
            slot_samples[(stream, j)] = ranks
            pad = (max(int(lens[stream][r]) for r in ranks) + 31) // 32 * 32
            pad = max(pad, 64)
            if stream != 0:
                pad = min(pad, S)
            slot_rows[(stream, j)] = pad
            row0 = 0
            for n in _split_rows(pad):
                groups.append(dict(stream=stream, j=j, row0=row0, n=n))
                row0 += n

    off = 0
    for g in groups:
        g["off"] = off
        off += g["n"]
    total_cols = off
    chunks = []
    i = 0
    while i < len(groups):
        n = groups[i]["n"]
        ng = 1
        while (i + ng < len(groups) and groups[i + ng]["n"] == n
               and (ng + 1) * 4 * n <= 16384):
            ng += 1
        chunks.append(dict(g0=i, ng=ng, n=n, off=groups[i]["off"]))
        i += ng
    return slot_samples, slot_rows, groups, chunks, total_cols


def _build_core_inputs(core, slot_samples, groups, total_cols, inputs):
    """Per-core fp8 stream (128, 4*total_cols) plus per-(stream,sample) padded
    row counts (for the host-side z correction)."""
    node_num = np.asarray(inputs["tree_node_num"])
    offs = np.concatenate([[0], np.cumsum(node_num)])
    tanh_h = inputs["_tanh_h"]
    feats = {1: inputs["desc_anchor_feat"], 2: inputs["desc_neg_feat"]}
    lens = {1: np.asarray(inputs["desc_anchor_len"]),
            2: np.asarray(inputs["desc_neg_len"])}

    f8 = _np_f8()
    L = 4 * total_cols
    xt = np.zeros((128, L), f8)
    pad_counts = np.zeros((3, B), np.int64)

    for g in groups:
        stream, j, row0, n, off = g["stream"], g["j"], g["row0"], g["n"], g["off"]
        samp = slot_samples[(stream, j)][core]
        if stream == 0:
            n_real = int(node_num[samp])
            src = tanh_h[offs[samp]:offs[samp] + n_real]
        else:
            n_real = int(lens[stream][samp])
            src = feats[stream][samp]
        r1 = min(row0 + n, n_real)
        nvalid = max(r1 - row0, 0)
        block = np.zeros((n, H), np.float32)
        if nvalid:
            block[:nvalid] = src[row0:r1]
        pad_counts[stream, samp] += n - nvalid
        slab = block.astype(f8).T.reshape(4, 128, n).transpose(1, 0, 2)
        xt[:, 4 * off:4 * off + 4 * n] = slab.reshape(128, 4 * n)
    return xt, pad_counts


def _build_program(groups, chunks, total_cols, repeat=1,
                   stages=('mm', 'act', 'pool'), with_bias=False):
    import concourse.bass as bass
    import concourse.bacc as bacc
    import concourse.tile as tile
    from concourse import mybir

    f32 = mybir.dt.float32
    f8 = mybir.dt.float8e4
    DR = mybir.MatmulPerfMode.DoubleRow
    Tanh = mybir.ActivationFunctionType.Tanh
    Exp = mybir.ActivationFunctionType.Exp
    G = len(groups)
    L = 4 * total_cols

    nc = bacc.Bacc("TRN2", target_bir_lowering=False, debug=False)
    xt_d = nc.dram_tensor("xt", (128, L), f8, kind="ExternalInput")
    wt_d = nc.dram_tensor("wt2", (128, 2, 4, 2, 2, 128), f8, kind="ExternalInput")
    v_d = nc.dram_tensor("v2", (128, 2, 2, 2, 128), f8, kind="ExternalInput")
    bs_d = nc.dram_tensor("bias", (128, 2, 4), f32, kind="ExternalInput")
    pooled_d = nc.dram_tensor("pooled", (128, 4 * G), f32, kind="ExternalOutput")
    zs_d = nc.dram_tensor("zs", (1, G), f32, kind="ExternalOutput")

    def subs_of(n):
        if n <= 256:
            return ((0, n),)
        n1 = (n // 2 + 31) // 32 * 32
        return ((0, n1), (n1, n - n1))

    with tile.TileContext(nc) as tc:
        with (
            tc.tile_pool(name="const", bufs=1) as const,
            tc.tile_pool(name="io", bufs=1) as io,
            tc.tile_pool(name="ck_p", bufs=4) as ck_p,
            tc.tile_pool(name="ut_p", bufs=3) as ut_p,
            tc.tile_pool(name="e_p", bufs=3) as e_p,
            tc.tile_pool(name="scr_p", bufs=3) as scr_p,
            tc.tile_pool(name="psum", bufs=1, space="PSUM") as psum,
        ):
            w_sb = const.tile([128, 2, 4, 2, 2, 128], f8)
            nc.sync.dma_start(out=w_sb, in_=bass.AP(
                tensor=wt_d, offset=0, ap=[[4096, 128], [1, 4096]]))
            v_sb = const.tile([128, 2, 2, 2, 128], f8)
            nc.sync.dma_start(out=v_sb, in_=bass.AP(
                tensor=v_d, offset=0, ap=[[1024, 128], [1, 1024]]))
            b_sb = const.tile([128, 2, 4], f32)
            nc.sync.dma_start(out=b_sb, in_=bass.AP(
                tensor=bs_d, offset=0, ap=[[8, 128], [1, 8]]))

            pooled_sb = io.tile([128, 4 * G], f32)
            zcols = io.tile([128, G], f32)
            if stages != ('mm', 'act', 'pool'):
                nc.vector.memset(pooled_sb, 0.0)
                nc.vector.memset(zcols, 0.0)

            def emit_front(g, gi, ck, gj):
                """W matmuls + tanh for group g; returns state for emit_back.

                The W accumulator is two 2-bank PSUM halves (mc pairs) with
                bufs=3: tanh of half 0 overlaps the W matmuls of half 1, and
                the next group's W never waits on this group's v/exp (which
                use the separate `ps` tile). Each mc slice spans exactly one
                PSUM bank (matmul outputs must not straddle banks).
                """
                n = g["n"]
                widx = 0 if g["stream"] == 0 else 1
                if 'act' in stages:
                    ut = ut_p.tile([128, 4, n], f8, tag="ut",
                                   padded_shape=[128, 4, 512])
                for half in (0, 1):
                    pth = psum.tile([128, 2, 512], f32, tag="pth", bufs=3)
                    for kp in (0, 1):
                        for ml in (0, 1):
                            mc = 2 * half + ml
                            for (o, ns) in subs_of(n):
                                nc.tensor.matmul(
                                    pth[:, ml, o:o + ns],
                                    lhsT=w_sb[:, widx, mc, kp, :, :],
                                    rhs=ck[:, gj, 2 * kp:2 * kp + 2, o:o + ns],
                                    start=(kp == 0), stop=(kp == 1),
                                    perf_mode=DR)
                    if 'act' not in stages:
                        continue
                    if with_bias:
                        for ml in (0, 1):
                            mc = 2 * half + ml
                            nc.scalar.activation(
                                out=ut[:, mc, :], in_=pth[:, ml, 0:n],
                                func=Tanh, bias=b_sb[:, widx, mc:mc + 1],
                                scale=1.0)
                    else:
                        nc.scalar.activation(
                            out=ut[:, 2 * half:2 * half + 2, :],
                            in_=pth[:, :, 0:n], func=Tanh)
                if 'act' not in stages:
                    return None
                return (g, gi, ck, gj, ut)

            def emit_back(state):
                """v matmuls + exp + pooling for a group emitted earlier."""
                if state is None:
                    return
                g, gi, ck, gj, ut = state
                n = g["n"]
                widx = 0 if g["stream"] == 0 else 1
                ps = psum.tile([128, 512], f32, tag="ps", bufs=2)
                for kp in (0, 1):
                    for (o, ns) in subs_of(n):
                        nc.tensor.matmul(
                            ps[:, o:o + ns],
                            lhsT=v_sb[:, widx, kp, :, :],
                            rhs=ut[:, 2 * kp:2 * kp + 2, o:o + ns],
                            start=(kp == 0), stop=(kp == 1),
                            perf_mode=DR)
                e_t = e_p.tile([128, n], f8, tag="e", padded_shape=[128, 512])
                nc.scalar.activation(out=e_t, in_=ps[:, 0:n], func=Exp,
                                     accum_out=zcols[:, gi:gi + 1])
                if 'pool' not in stages:
                    return
                for c in range(4):
                    scr = scr_p.tile([128, n], f8, tag="scr",
                                     padded_shape=[128, 512])
                    nc.vector.scalar_tensor_tensor(
                        out=scr, in0=ck[:, gj, c, :], scalar=1.0, in1=e_t,
                        op0=mybir.AluOpType.mult, op1=mybir.AluOpType.mult,
                        accum_out=pooled_sb[:, 4 * gi + c:4 * gi + c + 1])

            loop_cm = (tc.For_i(0, repeat, 1) if repeat > 1
                       else contextlib.nullcontext())
            with loop_cm:
                pending = None
                for ch in chunks:
                    n, ng = ch["n"], ch["ng"]
                    ck_flat = ck_p.tile([128, ng * 4 * n], f8, tag="ck",
                                        padded_shape=[128, 16384])
                    nc.sync.dma_start(out=ck_flat, in_=bass.AP(
                        tensor=xt_d, offset=4 * ch["off"],
                        ap=[[L, 128], [1, ng * 4 * n]]))
                    ck = ck_flat.rearrange("p (g k n) -> p g k n", g=ng, k=4)
                    if 'mm' not in stages:
                        continue
                    for gj in range(ng):
                        gi = ch["g0"] + gj
                        st = emit_front(groups[gi], gi, ck, gj)
                        emit_back(pending)
                        pending = st
                emit_back(pending)

            nc.sync.dma_start(out=pooled_d.ap(), in_=pooled_sb)
            nc.sync.dma_start(out=zs_d.ap(), in_=zcols[0:1, :])

    nc.compile()
    return nc


def _run_spmd(nc, in_maps):
    """SPMD-execute `nc` on 8 cores via PJRT, caching the jitted executable and
    keeping the big inputs device-resident so repeated runs can be timed."""
    import jax
    import numpy as np_
    from jax.experimental.shard_map import shard_map
    from jax.sharding import Mesh, NamedSharding, PartitionSpec
    from concourse import mybir
    from concourse.bass2jax import (_bass_exec_p, install_neuronx_cc_hook,
                                    partition_id_tensor)

    n_cores = len(in_maps)
    st = _RUN_STATE.get(id(nc))
    if st is None:
        install_neuronx_cc_hook()
        partition_name = (nc.partition_id_tensor.name
                          if nc.partition_id_tensor else None)
        in_names, out_names, out_avals = [], [], []
        for alloc in nc.m.functions[0].allocations:
            if not isinstance(alloc, mybir.MemoryLocationSet):
                continue
            name = alloc.memorylocations[0].name
            if alloc.kind == "ExternalInput":
                if name != partition_name:
                    in_names.append(name)
            elif alloc.kind == "ExternalOutput":
                out_names.append(name)
                out_avals.append(jax.core.ShapedArray(
                    tuple(alloc.tensor_shape), mybir.dt.np(alloc.dtype)))
        n_params = len(in_names)
        all_names = in_names + out_names
        if partition_name is not None:
            all_names = all_names + [partition_name]
        donate = tuple(range(n_params, n_params + len(out_names)))

        def _body(*args):
            operands = list(args)
            if partition_name is not None:
                operands.append(partition_id_tensor())
            return tuple(_bass_exec_p.bind(
                *operands, out_avals=tuple(out_avals), in_names=tuple(all_names),
                out_names=tuple(out_names), lowering_input_output_aliases=(),
                sim_require_finite=True, sim_require_nnan=True, nc=nc))

        devices = jax.devices()[:n_cores]
        mesh = Mesh(np_.asarray(devices), ("core",))
        in_specs = (PartitionSpec("core"),) * (n_params + len(out_names))
        out_specs = (PartitionSpec("core"),) * len(out_names)
        sharded = jax.jit(
            shard_map(_body, mesh=mesh, in_specs=in_specs,
                      out_specs=out_specs, check_rep=False),
            donate_argnums=donate, keep_unused=True)
        st = dict(sharded=sharded, mesh=mesh, in_names=in_names,
                  out_names=out_names, out_avals=out_avals, n_cores=n_cores)
        _RUN_STATE[id(nc)] = st

    sharding = NamedSharding(st["mesh"], PartitionSpec("core"))
    concat_in = [
        np_.concatenate([np_.asarray(m[name]) for m in in_maps], axis=0)
        for name in st["in_names"]]
    st["resident_in"] = [jax.device_put(a, sharding) for a in concat_in]
    for a in st["resident_in"]:
        a.block_until_ready()
    out_arrs = _exec_once(st)
    results = [
        {name: np_.asarray(out_arrs[i]).reshape(
            st["n_cores"], *st["out_avals"][i].shape)[c]
         for i, name in enumerate(st["out_names"])}
        for c in range(st["n_cores"])]
    st["last_out"] = out_arrs
    return results


def _exec_once(st):
    import numpy as np_
    zeros = [np_.zeros((st["n_cores"] * av.shape[0], *av.shape[1:]), av.dtype)
             for av in st["out_avals"]]
    return st["sharded"](*st["resident_in"], *zeros)


def benchmark(iters=10):
    """Time repeated executions of the last-run kernel (inputs resident on
    device). Returns per-iteration seconds (min over runs)."""
    import time
    st = _RUN_STATE.get("_main_st")
    assert st is not None and "resident_in" in st, "run kernel() first"
    _exec_once(st)[-1].block_until_ready()  # warm
    times = []
    for _ in range(3):
        t0 = time.perf_counter()
        outs = None
        for _ in range(iters):
            outs = _exec_once(st)
        for o in outs:
            o.block_until_ready()
        times.append((time.perf_counter() - t0) / iters)
    return min(times)


def benchmark_slope(r1=132, r2=260, reps=5):
    """True on-device per-iteration seconds, immune to host/tunnel dispatch
    overhead: runs repeat-loop variants of the last planned program and
    returns (wall(r2)-wall(r1))/(r2-r1)."""
    import time
    plan = _RUN_STATE.get("_last_plan")
    assert plan is not None, "run kernel() first"
    groups, chunks, total_cols, with_bias, in_maps = plan
    walls = {}
    for R in (r1, r2):
        key = ("slope", R, total_cols, with_bias, len(groups))
        nc = _PROGRAM_CACHE.get(key)
        if nc is None:
            nc = _build_program(groups, chunks, total_cols, repeat=R,
                                with_bias=with_bias)
            _PROGRAM_CACHE[key] = nc
        _run_spmd(nc, in_maps)
        st = _RUN_STATE[id(nc)]
        ws = []
        for _ in range(reps):
            t0 = time.perf_counter()
            outs = _exec_once(st)
            for o in outs:
                o.block_until_ready()
            ws.append(time.perf_counter() - t0)
        walls[R] = min(ws)
    return (walls[r2] - walls[r1]) / (r2 - r1)


def kernel(all_node_h, tree_node_num, desc_anchor_feat, desc_anchor_len,
           desc_neg_feat, desc_neg_len, W1, b1, v1, v1b, W2, b2, v2, v2b):
    global LAST_RESULTS
    f8 = _np_f8()

    inputs = dict(
        tree_node_num=np.asarray(tree_node_num),
        desc_anchor_feat=np.asarray(desc_anchor_feat, np.float32),
        desc_anchor_len=np.asarray(desc_anchor_len),
        desc_neg_feat=np.asarray(desc_neg_feat, np.float32),
        desc_neg_len=np.asarray(desc_neg_len),
        _tanh_h=np.tanh(np.asarray(all_node_h, np.float32)))
    slot_samples, slot_rows, groups, chunks, total_cols = _plan(
        inputs["tree_node_num"], inputs["desc_anchor_len"],
        inputs["desc_neg_len"])
    G = len(groups)

    # weights: wt2[k, w, mc, kp, i, m] = W_w[128*(2kp+i)+k, 128*mc+m]
    Wq = np.stack([W1, W2]).astype(np.float32)
    wt2 = np.ascontiguousarray(
        Wq.reshape(2, 2, 2, 128, 4, 128).transpose(3, 0, 4, 1, 2, 5)
        .astype(f8))
    vq = np.stack([v1, v2]).astype(np.float32)
    v2a = np.ascontiguousarray(np.broadcast_to(
        vq.reshape(2, 2, 2, 128).transpose(3, 0, 1, 2)[..., None],
        (128, 2, 2, 2, 128)).astype(f8))
    bsq = np.stack([b1, b2]).astype(np.float32)
    with_bias = bool(np.any(bsq != 0.0))
    bs = np.ascontiguousarray(bsq.reshape(2, 4, 128).transpose(2, 0, 1))
    # z contribution of an all-zero padded row: exp(v . tanh(b)) per stream
    # (fp8-quantized the same way the device computes it)
    v_q = vq.astype(f8).astype(np.float32)
    b_q = np.tanh(bsq).astype(f8).astype(np.float32)
    pad_z = np.exp((v_q * b_q).sum(axis=1))        # (2,)
    pad_z = np.array([pad_z[0], pad_z[1], pad_z[1]])

    in_maps, pad_counts = [], []
    for core in range(NCORES):
        xt, pc = _build_core_inputs(core, slot_samples, groups, total_cols,
                                    inputs)
        in_maps.append({"xt": xt, "wt2": wt2, "v2": v2a, "bias": bs})
        pad_counts.append(pc)

    cache_key = (total_cols, with_bias) + tuple(
        (g["stream"], g["n"]) for g in groups)
    nc = _PROGRAM_CACHE.get(cache_key)
    if nc is None:
        nc = _build_program(groups, chunks, total_cols, with_bias=with_bias)
        _PROGRAM_CACHE[cache_key] = nc

    results = _run_spmd(nc, in_maps)
    _RUN_STATE["_main_st"] = _RUN_STATE[id(nc)]
    _RUN_STATE["_last_plan"] = (groups, chunks, total_cols, with_bias, in_maps)

    # ---- host finalization (f64, trivially small) ----
    pooled_acc = np.zeros((3, B, H), np.float64)
    z_acc = np.zeros((3, B), np.float64)
    for core in range(NCORES):
        out = results[core]
        pooled = np.asarray(out["pooled"], np.float64)   # (128, 4G)
        zs = np.asarray(out["zs"], np.float64).reshape(G)
        for gi, g in enumerate(groups):
            samp = slot_samples[(g["stream"], g["j"])][core]
            vec = pooled[:, 4 * gi:4 * gi + 4].T.reshape(H)
            pooled_acc[g["stream"], samp] += vec
            z_acc[g["stream"], samp] += zs[gi]
        z_acc -= pad_counts[core] * pad_z[:, None]

    reprs = np.tanh(pooled_acc / z_acc[:, :, None])
    code, anc, neg = reprs

    def cos(x, y):
        num = (x * y).sum(axis=1)
        den = np.linalg.norm(x, axis=1) * np.linalg.norm(y, axis=1) + EPS
        return num / den

    loss = np.mean(np.clip(MARGIN - cos(code, anc) + cos(code, neg), 1e-6, None))
    return np.float32(loss)


# revision 12
# speedup vs baseline: 1.0052x; 1.0052x over previous
"""Trainium2 Bass kernel for nn_ASTEmbeder (AST code/desc attention-pool + hinge loss).

V2 strategy (data-parallel over batch, 8 cores, fp8 everywhere):
- Samples are slotted so every core runs one SPMD program: code trees sorted
  by size, desc streams sorted by length (independently per stream), slot j =
  sorted ranks [8j, 8j+8). Slot width = max len in slot (64-padded), so short
  descriptions are never loaded at all.
- Host prep: h = tanh(all_node_h) on host; streams quantized to fp8e4m3,
  transposed (H on partitions), one contiguous (128, L) byte stream per core
  -> 10-16KB/partition chunk DMAs.
- No masks: padded rows are zeroed in the stream. A zero row contributes 0 to
  the pooled sum and exactly exp(v.tanh(b)) (=1 for b=0) to z; the host
  subtracts pad_count * exp(v.tanh(b)) from z.
- On-chip per group of n<=512 cols (rows of the original layout):
    t = W.T @ h.T      16 fp8 DoubleRow matmuls (256-wide K pairs) into one
                       4-bank PSUM tile
    u = tanh(t)        1 ACT instr over all 4 banks, fp8 out (b==0 fast path)
    s = v.T @ u        4 fp8 DoubleRow matmuls, v replicated over 128 cols
    e = exp(s)         ACT, fp8 out, z via accum_out
    pooled[mc] += ...  4 DVE scalar_tensor_tensor with f32 accum columns
- Emission is software-pipelined: group g's v/exp/pool instructions are
  emitted after group g+1's W/tanh, so PE never waits on ACT.
- Host: repr = tanh(pooled/z), cosine sims, hinge loss. Softmax
  max-subtraction dropped (logits O(1)); v1b/v2b dropped (shift-invariance).
"""
import contextlib
import os
import numpy as np

B, H, S = 256, 512, 512
NCORES = 8
MARGIN, EPS = 0.05, 1e-8

LAST_RESULTS = None
_PROGRAM_CACHE = {}
_RUN_STATE = {}  # cached jitted executable + device-resident inputs (timing)


def _np_f8():
    import ml_dtypes
    return ml_dtypes.float8_e4m3fn


def _split_rows(n, cap=512):
    """Split n rows into near-equal groups <= cap, multiples of 32 except the
    last."""
    if n <= cap:
        return [n]
    k = (n + cap - 1) // cap
    base = (n // k + 31) // 32 * 32
    out, left = [], n
    for _ in range(k - 1):
        out.append(base)
        left -= base
    out.append(left)
    return out


def _plan(node_num, anc_len, neg_len):
    """Slot assignment + group/chunk plan shared by all cores (SPMD).

    Every stream independently sorts its samples so slot j holds sorted ranks
    [8j, 8j+8) (core k takes rank 8j+k); slot width = 64-padded max length.

    Returns (slot_samples, slot_rows, groups, chunks, total_cols) where
      slot_samples[(stream, j)][k] = sample id for core k
      slot_rows[(stream, j)] = padded row count of the slot
      groups: dicts(stream, j, row0, n, off)
      chunks: dicts(g0, ng, n, off)  -- ng consecutive groups of equal n
    """
    lens = {0: np.asarray(node_num), 1: np.asarray(anc_len),
            2: np.asarray(neg_len)}
    nslots = B // NCORES
    slot_samples, slot_rows = {}, {}
    groups = []
    for stream in (0, 1, 2):
        order = np.argsort(lens[stream], kind="stable")
        for j in range(nslots):
            ranks = [int(r) for r in order[j * NCORES:(j + 1) * NCORES]]
            slot_samples[(stream, j)] = ranks
            pad = (max(int(lens[stream][r]) for r in ranks) + 63) // 64 * 64
            pad = max(pad, 64)
            if stream != 0:
                pad = min(pad, S)
            slot_rows[(stream, j)] = pad
            row0 = 0
            for n in _split_rows(pad):
                groups.append(dict(stream=stream, j=j, row0=row0, n=n))
                row0 += n

    off = 0
    for g in groups:
        g["off"] = off
        off += g["n"]
    total_cols = off
    chunks = []
    i = 0
    while i < len(groups):
        n = groups[i]["n"]
        ng = 1
        while (i + ng < len(groups) and groups[i + ng]["n"] == n
               and (ng + 1) * 4 * n <= 16384):
            ng += 1
        chunks.append(dict(g0=i, ng=ng, n=n, off=groups[i]["off"]))
        i += ng
    return slot_samples, slot_rows, groups, chunks, total_cols


def _build_core_inputs(core, slot_samples, groups, total_cols, inputs):
    """Per-core fp8 stream (128, 4*total_cols) plus per-(stream,sample) padded
    row counts (for the host-side z correction)."""
    node_num = np.asarray(inputs["tree_node_num"])
    offs = np.concatenate([[0], np.cumsum(node_num)])
    tanh_h = inputs["_tanh_h"]
    feats = {1: inputs["desc_anchor_feat"], 2: inputs["desc_neg_feat"]}
    lens = {1: np.asarray(inputs["desc_anchor_len"]),
            2: np.asarray(inputs["desc_neg_len"])}

    f8 = _np_f8()
    L = 4 * total_cols
    xt = np.zeros((128, L), f8)
    pad_counts = np.zeros((3, B), np.int64)

    for g in groups:
        stream, j, row0, n, off = g["stream"], g["j"], g["row0"], g["n"], g["off"]
        samp = slot_samples[(stream, j)][core]
        if stream == 0:
            n_real = int(node_num[samp])
            src = tanh_h[offs[samp]:offs[samp] + n_real]
        else:
            n_real = int(lens[stream][samp])
            src = feats[stream][samp]
        r1 = min(row0 + n, n_real)
        nvalid = max(r1 - row0, 0)
        block = np.zeros((n, H), np.float32)
        if nvalid:
            block[:nvalid] = src[row0:r1]
        pad_counts[stream, samp] += n - nvalid
        slab = block.astype(f8).T.reshape(4, 128, n).transpose(1, 0, 2)
        xt[:, 4 * off:4 * off + 4 * n] = slab.reshape(128, 4 * n)
    return xt, pad_counts


def _build_program(groups, chunks, total_cols, repeat=1,
                   stages=('mm', 'act', 'pool'), with_bias=False):
    import concourse.bass as bass
    import concourse.bacc as bacc
    import concourse.tile as tile
    from concourse import mybir

    f32 = mybir.dt.float32
    f8 = mybir.dt.float8e4
    DR = mybir.MatmulPerfMode.DoubleRow
    Tanh = mybir.ActivationFunctionType.Tanh
    Exp = mybir.ActivationFunctionType.Exp
    G = len(groups)
    L = 4 * total_cols

    nc = bacc.Bacc("TRN2", target_bir_lowering=False, debug=False)
    xt_d = nc.dram_tensor("xt", (128, L), f8, kind="ExternalInput")
    wt_d = nc.dram_tensor("wt2", (128, 2, 4, 2, 2, 128), f8, kind="ExternalInput")
    v_d = nc.dram_tensor("v2", (128, 2, 2, 2, 128), f8, kind="ExternalInput")
    bs_d = nc.dram_tensor("bias", (128, 2, 4), f32, kind="ExternalInput")
    pooled_d = nc.dram_tensor("pooled", (128, 4 * G), f32, kind="ExternalOutput")
    zs_d = nc.dram_tensor("zs", (1, G), f32, kind="ExternalOutput")

    def subs_of(n):
        if n <= 256:
            return ((0, n),)
        n1 = (n // 2 + 31) // 32 * 32
        return ((0, n1), (n1, n - n1))

    with tile.TileContext(nc) as tc:
        with (
            tc.tile_pool(name="const", bufs=1) as const,
            tc.tile_pool(name="io", bufs=1) as io,
            tc.tile_pool(name="ck_p", bufs=4) as ck_p,
            tc.tile_pool(name="ut_p", bufs=3) as ut_p,
            tc.tile_pool(name="e_p", bufs=3) as e_p,
            tc.tile_pool(name="scr_p", bufs=3) as scr_p,
            tc.tile_pool(name="psum", bufs=1, space="PSUM") as psum,
        ):
            w_sb = const.tile([128, 2, 4, 2, 2, 128], f8)
            nc.sync.dma_start(out=w_sb, in_=bass.AP(
                tensor=wt_d, offset=0, ap=[[4096, 128], [1, 4096]]))
            v_sb = const.tile([128, 2, 2, 2, 128], f8)
            nc.sync.dma_start(out=v_sb, in_=bass.AP(
                tensor=v_d, offset=0, ap=[[1024, 128], [1, 1024]]))
            b_sb = const.tile([128, 2, 4], f32)
            nc.sync.dma_start(out=b_sb, in_=bass.AP(
                tensor=bs_d, offset=0, ap=[[8, 128], [1, 8]]))

            pooled_sb = io.tile([128, 4 * G], f32)
            zcols = io.tile([128, G], f32)
            if stages != ('mm', 'act', 'pool'):
                nc.vector.memset(pooled_sb, 0.0)
                nc.vector.memset(zcols, 0.0)

            def emit_front(g, gi, ck, gj):
                """W matmuls + tanh for group g; returns state for emit_back.

                The W accumulator is two 2-bank PSUM halves (mc pairs) with
                bufs=3: tanh of half 0 overlaps the W matmuls of half 1, and
                the next group's W never waits on this group's v/exp (which
                use the separate `ps` tile). Each mc slice spans exactly one
                PSUM bank (matmul outputs must not straddle banks).
                """
                n = g["n"]
                widx = 0 if g["stream"] == 0 else 1
                if 'act' in stages:
                    ut = ut_p.tile([128, 4, n], f8, tag="ut",
                                   padded_shape=[128, 4, 512])
                for half in (0, 1):
                    pth = psum.tile([128, 2, 512], f32, tag="pth", bufs=3)
                    for kp in (0, 1):
                        for ml in (0, 1):
                            mc = 2 * half + ml
                            for (o, ns) in subs_of(n):
                                nc.tensor.matmul(
                                    pth[:, ml, o:o + ns],
                                    lhsT=w_sb[:, widx, mc, kp, :, :],
                                    rhs=ck[:, gj, 2 * kp:2 * kp + 2, o:o + ns],
                                    start=(kp == 0), stop=(kp == 1),
                                    perf_mode=DR)
                    if 'act' not in stages:
                        continue
                    if with_bias:
                        for ml in (0, 1):
                            mc = 2 * half + ml
                            nc.scalar.activation(
                                out=ut[:, mc, :], in_=pth[:, ml, 0:n],
                                func=Tanh, bias=b_sb[:, widx, mc:mc + 1],
                                scale=1.0)
                    else:
                        nc.scalar.activation(
                            out=ut[:, 2 * half:2 * half + 2, :],
                            in_=pth[:, :, 0:n], func=Tanh)
                if 'act' not in stages:
                    return None
                return (g, gi, ck, gj, ut)

            def emit_back(state):
                """v matmuls + exp + pooling for a group emitted earlier."""
                if state is None:
                    return
                g, gi, ck, gj, ut = state
                n = g["n"]
                widx = 0 if g["stream"] == 0 else 1
                ps = psum.tile([128, 512], f32, tag="ps", bufs=2)
                for kp in (0, 1):
                    for (o, ns) in subs_of(n):
                        nc.tensor.matmul(
                            ps[:, o:o + ns],
                            lhsT=v_sb[:, widx, kp, :, :],
                            rhs=ut[:, 2 * kp:2 * kp + 2, o:o + ns],
                            start=(kp == 0), stop=(kp == 1),
                            perf_mode=DR)
                e_t = e_p.tile([128, n], f8, tag="e", padded_shape=[128, 512])
                nc.scalar.activation(out=e_t, in_=ps[:, 0:n], func=Exp,
                                     accum_out=zcols[:, gi:gi + 1])
                if 'pool' not in stages:
                    return
                for c in range(4):
                    scr = scr_p.tile([128, n], f8, tag="scr",
                                     padded_shape=[128, 512])
                    nc.vector.scalar_tensor_tensor(
                        out=scr, in0=ck[:, gj, c, :], scalar=1.0, in1=e_t,
                        op0=mybir.AluOpType.mult, op1=mybir.AluOpType.mult,
                        accum_out=pooled_sb[:, 4 * gi + c:4 * gi + c + 1])

            loop_cm = (tc.For_i(0, repeat, 1) if repeat > 1
                       else contextlib.nullcontext())
            with loop_cm:
                pending = None
                for ch in chunks:
                    n, ng = ch["n"], ch["ng"]
                    ck_flat = ck_p.tile([128, ng * 4 * n], f8, tag="ck",
                                        padded_shape=[128, 16384])
                    nc.sync.dma_start(out=ck_flat, in_=bass.AP(
                        tensor=xt_d, offset=4 * ch["off"],
                        ap=[[L, 128], [1, ng * 4 * n]]))
                    ck = ck_flat.rearrange("p (g k n) -> p g k n", g=ng, k=4)
                    if 'mm' not in stages:
                        continue
                    for gj in range(ng):
                        gi = ch["g0"] + gj
                        st = emit_front(groups[gi], gi, ck, gj)
                        emit_back(pending)
                        pending = st
                emit_back(pending)

            nc.sync.dma_start(out=pooled_d.ap(), in_=pooled_sb)
            nc.sync.dma_start(out=zs_d.ap(), in_=zcols[0:1, :])

    nc.compile()
    return nc


def _run_spmd(nc, in_maps):
    """SPMD-execute `nc` on 8 cores via PJRT, caching the jitted executable and
    keeping the big inputs device-resident so repeated runs can be timed."""
    import jax
    import numpy as np_
    from jax.experimental.shard_map import shard_map
    from jax.sharding import Mesh, NamedSharding, PartitionSpec
    from concourse import mybir
    from concourse.bass2jax import (_bass_exec_p, install_neuronx_cc_hook,
                                    partition_id_tensor)

    n_cores = len(in_maps)
    st = _RUN_STATE.get(id(nc))
    if st is None:
        install_neuronx_cc_hook()
        partition_name = (nc.partition_id_tensor.name
                          if nc.partition_id_tensor else None)
        in_names, out_names, out_avals = [], [], []
        for alloc in nc.m.functions[0].allocations:
            if not isinstance(alloc, mybir.MemoryLocationSet):
                continue
            name = alloc.memorylocations[0].name
            if alloc.kind == "ExternalInput":
                if name != partition_name:
                    in_names.append(name)
            elif alloc.kind == "ExternalOutput":
                out_names.append(name)
                out_avals.append(jax.core.ShapedArray(
                    tuple(alloc.tensor_shape), mybir.dt.np(alloc.dtype)))
        n_params = len(in_names)
        all_names = in_names + out_names
        if partition_name is not None:
            all_names = all_names + [partition_name]
        donate = tuple(range(n_params, n_params + len(out_names)))

        def _body(*args):
            operands = list(args)
            if partition_name is not None:
                operands.append(partition_id_tensor())
            return tuple(_bass_exec_p.bind(
                *operands, out_avals=tuple(out_avals), in_names=tuple(all_names),
                out_names=tuple(out_names), lowering_input_output_aliases=(),
                sim_require_finite=True, sim_require_nnan=True, nc=nc))

        devices = jax.devices()[:n_cores]
        mesh = Mesh(np_.asarray(devices), ("core",))
        in_specs = (PartitionSpec("core"),) * (n_params + len(out_names))
        out_specs = (PartitionSpec("core"),) * len(out_names)
        sharded = jax.jit(
            shard_map(_body, mesh=mesh, in_specs=in_specs,
                      out_specs=out_specs, check_rep=False),
            donate_argnums=donate, keep_unused=True)
        st = dict(sharded=sharded, mesh=mesh, in_names=in_names,
                  out_names=out_names, out_avals=out_avals, n_cores=n_cores)
        _RUN_STATE[id(nc)] = st

    sharding = NamedSharding(st["mesh"], PartitionSpec("core"))
    concat_in = [
        np_.concatenate([np_.asarray(m[name]) for m in in_maps], axis=0)
        for name in st["in_names"]]
    st["resident_in"] = [jax.device_put(a, sharding) for a in concat_in]
    for a in st["resident_in"]:
        a.block_until_ready()
    out_arrs = _exec_once(st)
    results = [
        {name: np_.asarray(out_arrs[i]).reshape(
            st["n_cores"], *st["out_avals"][i].shape)[c]
         for i, name in enumerate(st["out_names"])}
        for c in range(st["n_cores"])]
    st["last_out"] = out_arrs
    return results


def _exec_once(st):
    import numpy as np_
    zeros = [np_.zeros((st["n_cores"] * av.shape[0], *av.shape[1:]), av.dtype)
             for av in st["out_avals"]]
    return st["sharded"](*st["resident_in"], *zeros)


def benchmark(iters=10):
    """Time repeated executions of the last-run kernel (inputs resident on
    device). Returns per-iteration seconds (min over runs)."""
    import time
    st = _RUN_STATE.get("_main_st")
    assert st is not None and "resident_in" in st, "run kernel() first"
    _exec_once(st)[-1].block_until_ready()  # warm
    times = []
    for _ in range(3):
        t0 = time.perf_counter()
        outs = None
        for _ in range(iters):
            outs = _exec_once(st)
        for o in outs:
            o.block_until_ready()
        times.append((time.perf_counter() - t0) / iters)
    return min(times)


def benchmark_slope(r1=132, r2=260, reps=5):
    """True on-device per-iteration seconds, immune to host/tunnel dispatch
    overhead: runs repeat-loop variants of the last planned program and
    returns (wall(r2)-wall(r1))/(r2-r1)."""
    import time
    plan = _RUN_STATE.get("_last_plan")
    assert plan is not None, "run kernel() first"
    groups, chunks, total_cols, with_bias, in_maps = plan
    walls = {}
    for R in (r1, r2):
        key = ("slope", R, total_cols, with_bias, len(groups))
        nc = _PROGRAM_CACHE.get(key)
        if nc is None:
            nc = _build_program(groups, chunks, total_cols, repeat=R,
                                with_bias=with_bias)
            _PROGRAM_CACHE[key] = nc
        _run_spmd(nc, in_maps)
        st = _RUN_STATE[id(nc)]
        ws = []
        for _ in range(reps):
            t0 = time.perf_counter()
            outs = _exec_once(st)
            for o in outs:
                o.block_until_ready()
            ws.append(time.perf_counter() - t0)
        walls[R] = min(ws)
    return (walls[r2] - walls[r1]) / (r2 - r1)


def kernel(all_node_h, tree_node_num, desc_anchor_feat, desc_anchor_len,
           desc_neg_feat, desc_neg_len, W1, b1, v1, v1b, W2, b2, v2, v2b):
    global LAST_RESULTS
    f8 = _np_f8()

    inputs = dict(
        tree_node_num=np.asarray(tree_node_num),
        desc_anchor_feat=np.asarray(desc_anchor_feat, np.float32),
        desc_anchor_len=np.asarray(desc_anchor_len),
        desc_neg_feat=np.asarray(desc_neg_feat, np.float32),
        desc_neg_len=np.asarray(desc_neg_len),
        _tanh_h=np.tanh(np.asarray(all_node_h, np.float32)))
    slot_samples, slot_rows, groups, chunks, total_cols = _plan(
        inputs["tree_node_num"], inputs["desc_anchor_len"],
        inputs["desc_neg_len"])
    G = len(groups)

    # weights: wt2[k, w, mc, kp, i, m] = W_w[128*(2kp+i)+k, 128*mc+m]
    Wq = np.stack([W1, W2]).astype(np.float32)
    wt2 = np.ascontiguousarray(
        Wq.reshape(2, 2, 2, 128, 4, 128).transpose(3, 0, 4, 1, 2, 5)
        .astype(f8))
    vq = np.stack([v1, v2]).astype(np.float32)
    v2a = np.ascontiguousarray(np.broadcast_to(
        vq.reshape(2, 2, 2, 128).transpose(3, 0, 1, 2)[..., None],
        (128, 2, 2, 2, 128)).astype(f8))
    bsq = np.stack([b1, b2]).astype(np.float32)
    with_bias = bool(np.any(bsq != 0.0))
    bs = np.ascontiguousarray(bsq.reshape(2, 4, 128).transpose(2, 0, 1))
    # z contribution of an all-zero padded row: exp(v . tanh(b)) per stream
    # (fp8-quantized the same way the device computes it)
    v_q = vq.astype(f8).astype(np.float32)
    b_q = np.tanh(bsq).astype(f8).astype(np.float32)
    pad_z = np.exp((v_q * b_q).sum(axis=1))        # (2,)
    pad_z = np.array([pad_z[0], pad_z[1], pad_z[1]])

    in_maps, pad_counts = [], []
    for core in range(NCORES):
        xt, pc = _build_core_inputs(core, slot_samples, groups, total_cols,
                                    inputs)
        in_maps.append({"xt": xt, "wt2": wt2, "v2": v2a, "bias": bs})
        pad_counts.append(pc)

    cache_key = (total_cols, with_bias) + tuple(
        (g["stream"], g["n"]) for g in groups)
    nc = _PROGRAM_CACHE.get(cache_key)
    if nc is None:
        nc = _build_program(groups, chunks, total_cols, with_bias=with_bias)
        _PROGRAM_CACHE[cache_key] = nc

    results = _run_spmd(nc, in_maps)
    _RUN_STATE["_main_st"] = _RUN_STATE[id(nc)]
    _RUN_STATE["_last_plan"] = (groups, chunks, total_cols, with_bias, in_maps)

    # ---- host finalization (f64, trivially small) ----
    pooled_acc = np.zeros((3, B, H), np.float64)
    z_acc = np.zeros((3, B), np.float64)
    for core in range(NCORES):
        out = results[core]
        pooled = np.asarray(out["pooled"], np.float64)   # (128, 4G)
        zs = np.asarray(out["zs"], np.float64).reshape(G)
        for gi, g in enumerate(groups):
            samp = slot_samples[(g["stream"], g["j"])][core]
            vec = pooled[:, 4 * gi:4 * gi + 4].T.reshape(H)
            pooled_acc[g["stream"], samp] += vec
            z_acc[g["stream"], samp] += zs[gi]
        z_acc -= pad_counts[core] * pad_z[:, None]

    reprs = np.tanh(pooled_acc / z_acc[:, :, None])
    code, anc, neg = reprs

    def cos(x, y):
        num = (x * y).sum(axis=1)
        den = np.linalg.norm(x, axis=1) * np.linalg.norm(y, axis=1) + EPS
        return num / den

    loss = np.mean(np.clip(MARGIN - cos(code, anc) + cos(code, neg), 1e-6, None))
    return np.float32(loss)


# revision 13
# speedup vs baseline: 1.0200x; 1.0147x over previous
"""Trainium2 Bass kernel for nn_ASTEmbeder (AST code/desc attention-pool + hinge loss).

V2 strategy (data-parallel over batch, 8 cores, fp8 everywhere):
- Samples are slotted so every core runs one SPMD program: code trees sorted
  by size, desc streams sorted by length (independently per stream), slot j =
  sorted ranks [8j, 8j+8). Slot width = max len in slot (64-padded), so short
  descriptions are never loaded at all.
- Host prep: h = tanh(all_node_h) on host; streams quantized to fp8e4m3,
  transposed (H on partitions), one contiguous (128, L) byte stream per core
  -> 10-16KB/partition chunk DMAs.
- No masks: padded rows are zeroed in the stream. A zero row contributes 0 to
  the pooled sum and exactly exp(v.tanh(b)) (=1 for b=0) to z; the host
  subtracts pad_count * exp(v.tanh(b)) from z.
- On-chip per group of n<=512 cols (rows of the original layout):
    t = W.T @ h.T      16 fp8 DoubleRow matmuls (256-wide K pairs) into two
                       2-bank PSUM halves (bufs=3; each mc slice is exactly
                       one bank -- matmul outs must not straddle banks)
    u = tanh(t)        1 ACT instr per half, fp8 out (b==0 fast path)
    s = v.T @ u        4 fp8 DoubleRow matmuls into a separate 1-bank ps
                       (bufs=2), v replicated over 128 cols
    e = exp(s)         ACT, fp8 out, z via accum_out
    pooled[mc] += ...  4 DVE scalar_tensor_tensor with f32 accum columns
- Emission is software-pipelined: group g's v/exp/pool instructions are
  emitted after group g+1's W/tanh. With the halved W accumulator, tanh of
  half 0 overlaps W of half 1, and the separate ps tile means the next
  group's W never waits on this group's exp (PSUM = 3x2 + 2x1 = 8 banks).
- Host: repr = tanh(pooled/z), cosine sims, hinge loss. Softmax
  max-subtraction dropped (logits O(1)); v1b/v2b dropped (shift-invariance).
"""
import contextlib
import os
import numpy as np

B, H, S = 256, 512, 512
NCORES = 8
MARGIN, EPS = 0.05, 1e-8

LAST_RESULTS = None
_PROGRAM_CACHE = {}
_RUN_STATE = {}  # cached jitted executable + device-resident inputs (timing)


def _np_f8():
    import ml_dtypes
    return ml_dtypes.float8_e4m3fn


def _split_rows(n, cap=512):
    """Split n rows into near-equal groups <= cap, multiples of 32 except the
    last."""
    if n <= cap:
        return [n]
    k = (n + cap - 1) // cap
    base = (n // k + 31) // 32 * 32
    out, left = [], n
    for _ in range(k - 1):
        out.append(base)
        left -= base
    out.append(left)
    return out


def _plan(node_num, anc_len, neg_len):
    """Slot assignment + group/chunk plan shared by all cores (SPMD).

    Every stream independently sorts its samples so slot j holds sorted ranks
    [8j, 8j+8) (core k takes rank 8j+k); slot width = 64-padded max length.

    Returns (slot_samples, slot_rows, groups, chunks, total_cols) where
      slot_samples[(stream, j)][k] = sample id for core k
      slot_rows[(stream, j)] = padded row count of the slot
      groups: dicts(stream, j, row0, n, off)
      chunks: dicts(g0, ng, n, off)  -- ng consecutive groups of equal n
    """
    lens = {0: np.asarray(node_num), 1: np.asarray(anc_len),
            2: np.asarray(neg_len)}
    nslots = B // NCORES
    slot_samples, slot_rows = {}, {}
    groups = []
    for stream in (0, 1, 2):
        order = np.argsort(lens[stream], kind="stable")
        for j in range(nslots):
            ranks = [int(r) for r in order[j * NCORES:(j + 1) * NCORES]]
            slot_samples[(stream, j)] = ranks
            pad = (max(int(lens[stream][r]) for r in ranks) + 63) // 64 * 64
            pad = max(pad, 64)
            if stream != 0:
                pad = min(pad, S)
            slot_rows[(stream, j)] = pad
            row0 = 0
            for n in _split_rows(pad):
                groups.append(dict(stream=stream, j=j, row0=row0, n=n))
                row0 += n

    off = 0
    for g in groups:
        g["off"] = off
        off += g["n"]
    total_cols = off
    chunks = []
    i = 0
    while i < len(groups):
        n = groups[i]["n"]
        ng = 1
        while (i + ng < len(groups) and groups[i + ng]["n"] == n
               and (ng + 1) * 4 * n <= 16384):
            ng += 1
        chunks.append(dict(g0=i, ng=ng, n=n, off=groups[i]["off"]))
        i += ng
    return slot_samples, slot_rows, groups, chunks, total_cols


def _build_core_inputs(core, slot_samples, groups, total_cols, inputs):
    """Per-core fp8 stream (128, 4*total_cols) plus per-(stream,sample) padded
    row counts (for the host-side z correction)."""
    node_num = np.asarray(inputs["tree_node_num"])
    offs = np.concatenate([[0], np.cumsum(node_num)])
    tanh_h = inputs["_tanh_h"]
    feats = {1: inputs["desc_anchor_feat"], 2: inputs["desc_neg_feat"]}
    lens = {1: np.asarray(inputs["desc_anchor_len"]),
            2: np.asarray(inputs["desc_neg_len"])}

    f8 = _np_f8()
    L = 4 * total_cols
    xt = np.zeros((128, L), f8)
    pad_counts = np.zeros((3, B), np.int64)

    for g in groups:
        stream, j, row0, n, off = g["stream"], g["j"], g["row0"], g["n"], g["off"]
        samp = slot_samples[(stream, j)][core]
        if stream == 0:
            n_real = int(node_num[samp])
            src = tanh_h[offs[samp]:offs[samp] + n_real]
        else:
            n_real = int(lens[stream][samp])
            src = feats[stream][samp]
        r1 = min(row0 + n, n_real)
        nvalid = max(r1 - row0, 0)
        block = np.zeros((n, H), np.float32)
        if nvalid:
            block[:nvalid] = src[row0:r1]
        pad_counts[stream, samp] += n - nvalid
        slab = block.astype(f8).T.reshape(4, 128, n).transpose(1, 0, 2)
        xt[:, 4 * off:4 * off + 4 * n] = slab.reshape(128, 4 * n)
    return xt, pad_counts


def _build_program(groups, chunks, total_cols, repeat=1,
                   stages=('mm', 'act', 'pool'), with_bias=False):
    import concourse.bass as bass
    import concourse.bacc as bacc
    import concourse.tile as tile
    from concourse import mybir

    f32 = mybir.dt.float32
    f8 = mybir.dt.float8e4
    DR = mybir.MatmulPerfMode.DoubleRow
    Tanh = mybir.ActivationFunctionType.Tanh
    Exp = mybir.ActivationFunctionType.Exp
    G = len(groups)
    L = 4 * total_cols

    nc = bacc.Bacc("TRN2", target_bir_lowering=False, debug=False)
    xt_d = nc.dram_tensor("xt", (128, L), f8, kind="ExternalInput")
    wt_d = nc.dram_tensor("wt2", (128, 2, 4, 2, 2, 128), f8, kind="ExternalInput")
    v_d = nc.dram_tensor("v2", (128, 2, 2, 2, 128), f8, kind="ExternalInput")
    bs_d = nc.dram_tensor("bias", (128, 2, 4), f32, kind="ExternalInput")
    pooled_d = nc.dram_tensor("pooled", (128, 4 * G), f32, kind="ExternalOutput")
    zs_d = nc.dram_tensor("zs", (1, G), f32, kind="ExternalOutput")

    def subs_of(n):
        if n <= 256:
            return ((0, n),)
        n1 = (n // 2 + 31) // 32 * 32
        return ((0, n1), (n1, n - n1))

    with tile.TileContext(nc) as tc:
        with (
            tc.tile_pool(name="const", bufs=1) as const,
            tc.tile_pool(name="io", bufs=1) as io,
            tc.tile_pool(name="ck_p", bufs=4) as ck_p,
            tc.tile_pool(name="ut_p", bufs=3) as ut_p,
            tc.tile_pool(name="e_p", bufs=3) as e_p,
            tc.tile_pool(name="scr_p", bufs=3) as scr_p,
            tc.tile_pool(name="psum", bufs=1, space="PSUM") as psum,
        ):
            w_sb = const.tile([128, 2, 4, 2, 2, 128], f8)
            nc.sync.dma_start(out=w_sb, in_=bass.AP(
                tensor=wt_d, offset=0, ap=[[4096, 128], [1, 4096]]))
            v_sb = const.tile([128, 2, 2, 2, 128], f8)
            nc.sync.dma_start(out=v_sb, in_=bass.AP(
                tensor=v_d, offset=0, ap=[[1024, 128], [1, 1024]]))
            b_sb = const.tile([128, 2, 4], f32)
            nc.sync.dma_start(out=b_sb, in_=bass.AP(
                tensor=bs_d, offset=0, ap=[[8, 128], [1, 8]]))

            pooled_sb = io.tile([128, 4 * G], f32)
            zcols = io.tile([128, G], f32)
            if stages != ('mm', 'act', 'pool'):
                nc.vector.memset(pooled_sb, 0.0)
                nc.vector.memset(zcols, 0.0)

            def emit_front(g, gi, ck, gj):
                """W matmuls + tanh for group g; returns state for emit_back.

                The W accumulator is two 2-bank PSUM halves (mc pairs) with
                bufs=3: tanh of half 0 overlaps the W matmuls of half 1, and
                the next group's W never waits on this group's v/exp (which
                use the separate `ps` tile). Each mc slice spans exactly one
                PSUM bank (matmul outputs must not straddle banks).
                """
                n = g["n"]
                widx = 0 if g["stream"] == 0 else 1
                if 'act' in stages:
                    ut = ut_p.tile([128, 4, n], f8, tag="ut",
                                   padded_shape=[128, 4, 512])
                for half in (0, 1):
                    pth = psum.tile([128, 2, 512], f32, tag="pth", bufs=3)
                    for kp in (0, 1):
                        for ml in (0, 1):
                            mc = 2 * half + ml
                            for (o, ns) in subs_of(n):
                                nc.tensor.matmul(
                                    pth[:, ml, o:o + ns],
                                    lhsT=w_sb[:, widx, mc, kp, :, :],
                                    rhs=ck[:, gj, 2 * kp:2 * kp + 2, o:o + ns],
                                    start=(kp == 0), stop=(kp == 1),
                                    perf_mode=DR)
                    if 'act' not in stages:
                        continue
                    if with_bias:
                        for ml in (0, 1):
                            mc = 2 * half + ml
                            nc.scalar.activation(
                                out=ut[:, mc, :], in_=pth[:, ml, 0:n],
                                func=Tanh, bias=b_sb[:, widx, mc:mc + 1],
                                scale=1.0)
                    else:
                        nc.scalar.activation(
                            out=ut[:, 2 * half:2 * half + 2, :],
                            in_=pth[:, :, 0:n], func=Tanh)
                if 'act' not in stages:
                    return None
                return (g, gi, ck, gj, ut)

            def emit_back(state):
                """v matmuls + exp + pooling for a group emitted earlier."""
                if state is None:
                    return
                g, gi, ck, gj, ut = state
                n = g["n"]
                widx = 0 if g["stream"] == 0 else 1
                ps = psum.tile([128, 512], f32, tag="ps", bufs=2)
                for kp in (0, 1):
                    for (o, ns) in subs_of(n):
                        nc.tensor.matmul(
                            ps[:, o:o + ns],
                            lhsT=v_sb[:, widx, kp, :, :],
                            rhs=ut[:, 2 * kp:2 * kp + 2, o:o + ns],
                            start=(kp == 0), stop=(kp == 1),
                            perf_mode=DR)
                e_t = e_p.tile([128, n], f8, tag="e", padded_shape=[128, 512])
                nc.scalar.activation(out=e_t, in_=ps[:, 0:n], func=Exp,
                                     accum_out=zcols[:, gi:gi + 1])
                if 'pool' not in stages:
                    return
                for c in range(4):
                    scr = scr_p.tile([128, n], f8, tag="scr",
                                     padded_shape=[128, 512])
                    nc.vector.scalar_tensor_tensor(
                        out=scr, in0=ck[:, gj, c, :], scalar=1.0, in1=e_t,
                        op0=mybir.AluOpType.mult, op1=mybir.AluOpType.mult,
                        accum_out=pooled_sb[:, 4 * gi + c:4 * gi + c + 1])

            loop_cm = (tc.For_i(0, repeat, 1) if repeat > 1
                       else contextlib.nullcontext())
            with loop_cm:
                pending = None
                for ch in chunks:
                    n, ng = ch["n"], ch["ng"]
                    ck_flat = ck_p.tile([128, ng * 4 * n], f8, tag="ck",
                                        padded_shape=[128, 16384])
                    nc.sync.dma_start(out=ck_flat, in_=bass.AP(
                        tensor=xt_d, offset=4 * ch["off"],
                        ap=[[L, 128], [1, ng * 4 * n]]))
                    ck = ck_flat.rearrange("p (g k n) -> p g k n", g=ng, k=4)
                    if 'mm' not in stages:
                        continue
                    for gj in range(ng):
                        gi = ch["g0"] + gj
                        st = emit_front(groups[gi], gi, ck, gj)
                        emit_back(pending)
                        pending = st
                emit_back(pending)

            nc.sync.dma_start(out=pooled_d.ap(), in_=pooled_sb)
            nc.sync.dma_start(out=zs_d.ap(), in_=zcols[0:1, :])

    nc.compile()
    return nc


def _run_spmd(nc, in_maps):
    """SPMD-execute `nc` on 8 cores via PJRT, caching the jitted executable and
    keeping the big inputs device-resident so repeated runs can be timed."""
    import jax
    import numpy as np_
    from jax.experimental.shard_map import shard_map
    from jax.sharding import Mesh, NamedSharding, PartitionSpec
    from concourse import mybir
    from concourse.bass2jax import (_bass_exec_p, install_neuronx_cc_hook,
                                    partition_id_tensor)

    n_cores = len(in_maps)
    st = _RUN_STATE.get(id(nc))
    if st is None:
        install_neuronx_cc_hook()
        partition_name = (nc.partition_id_tensor.name
                          if nc.partition_id_tensor else None)
        in_names, out_names, out_avals = [], [], []
        for alloc in nc.m.functions[0].allocations:
            if not isinstance(alloc, mybir.MemoryLocationSet):
                continue
            name = alloc.memorylocations[0].name
            if alloc.kind == "ExternalInput":
                if name != partition_name:
                    in_names.append(name)
            elif alloc.kind == "ExternalOutput":
                out_names.append(name)
                out_avals.append(jax.core.ShapedArray(
                    tuple(alloc.tensor_shape), mybir.dt.np(alloc.dtype)))
        n_params = len(in_names)
        all_names = in_names + out_names
        if partition_name is not None:
            all_names = all_names + [partition_name]
        donate = tuple(range(n_params, n_params + len(out_names)))

        def _body(*args):
            operands = list(args)
            if partition_name is not None:
                operands.append(partition_id_tensor())
            return tuple(_bass_exec_p.bind(
                *operands, out_avals=tuple(out_avals), in_names=tuple(all_names),
                out_names=tuple(out_names), lowering_input_output_aliases=(),
                sim_require_finite=True, sim_require_nnan=True, nc=nc))

        devices = jax.devices()[:n_cores]
        mesh = Mesh(np_.asarray(devices), ("core",))
        in_specs = (PartitionSpec("core"),) * (n_params + len(out_names))
        out_specs = (PartitionSpec("core"),) * len(out_names)
        sharded = jax.jit(
            shard_map(_body, mesh=mesh, in_specs=in_specs,
                      out_specs=out_specs, check_rep=False),
            donate_argnums=donate, keep_unused=True)
        st = dict(sharded=sharded, mesh=mesh, in_names=in_names,
                  out_names=out_names, out_avals=out_avals, n_cores=n_cores)
        _RUN_STATE[id(nc)] = st

    sharding = NamedSharding(st["mesh"], PartitionSpec("core"))
    concat_in = [
        np_.concatenate([np_.asarray(m[name]) for m in in_maps], axis=0)
        for name in st["in_names"]]
    st["resident_in"] = [jax.device_put(a, sharding) for a in concat_in]
    for a in st["resident_in"]:
        a.block_until_ready()
    out_arrs = _exec_once(st)
    results = [
        {name: np_.asarray(out_arrs[i]).reshape(
            st["n_cores"], *st["out_avals"][i].shape)[c]
         for i, name in enumerate(st["out_names"])}
        for c in range(st["n_cores"])]
    st["last_out"] = out_arrs
    return results


def _exec_once(st):
    import numpy as np_
    zeros = [np_.zeros((st["n_cores"] * av.shape[0], *av.shape[1:]), av.dtype)
             for av in st["out_avals"]]
    return st["sharded"](*st["resident_in"], *zeros)


def benchmark(iters=10):
    """Time repeated executions of the last-run kernel (inputs resident on
    device). Returns per-iteration seconds (min over runs)."""
    import time
    st = _RUN_STATE.get("_main_st")
    assert st is not None and "resident_in" in st, "run kernel() first"
    _exec_once(st)[-1].block_until_ready()  # warm
    times = []
    for _ in range(3):
        t0 = time.perf_counter()
        outs = None
        for _ in range(iters):
            outs = _exec_once(st)
        for o in outs:
            o.block_until_ready()
        times.append((time.perf_counter() - t0) / iters)
    return min(times)


def benchmark_slope(r1=132, r2=260, reps=5):
    """True on-device per-iteration seconds, immune to host/tunnel dispatch
    overhead: runs repeat-loop variants of the last planned program and
    returns (wall(r2)-wall(r1))/(r2-r1)."""
    import time
    plan = _RUN_STATE.get("_last_plan")
    assert plan is not None, "run kernel() first"
    groups, chunks, total_cols, with_bias, in_maps = plan
    walls = {}
    for R in (r1, r2):
        key = ("slope", R, total_cols, with_bias, len(groups))
        nc = _PROGRAM_CACHE.get(key)
        if nc is None:
            nc = _build_program(groups, chunks, total_cols, repeat=R,
                                with_bias=with_bias)
            _PROGRAM_CACHE[key] = nc
        _run_spmd(nc, in_maps)
        st = _RUN_STATE[id(nc)]
        ws = []
        for _ in range(reps):
            t0 = time.perf_counter()
            outs = _exec_once(st)
            for o in outs:
                o.block_until_ready()
            ws.append(time.perf_counter() - t0)
        walls[R] = min(ws)
    return (walls[r2] - walls[r1]) / (r2 - r1)


def kernel(all_node_h, tree_node_num, desc_anchor_feat, desc_anchor_len,
           desc_neg_feat, desc_neg_len, W1, b1, v1, v1b, W2, b2, v2, v2b):
    global LAST_RESULTS
    f8 = _np_f8()

    inputs = dict(
        tree_node_num=np.asarray(tree_node_num),
        desc_anchor_feat=np.asarray(desc_anchor_feat, np.float32),
        desc_anchor_len=np.asarray(desc_anchor_len),
        desc_neg_feat=np.asarray(desc_neg_feat, np.float32),
        desc_neg_len=np.asarray(desc_neg_len),
        _tanh_h=np.tanh(np.asarray(all_node_h, np.float32)))
    slot_samples, slot_rows, groups, chunks, total_cols = _plan(
        inputs["tree_node_num"], inputs["desc_anchor_len"],
        inputs["desc_neg_len"])
    G = len(groups)

    # weights: wt2[k, w, mc, kp, i, m] = W_w[128*(2kp+i)+k, 128*mc+m]
    Wq = np.stack([W1, W2]).astype(np.float32)
    wt2 = np.ascontiguousarray(
        Wq.reshape(2, 2, 2, 128, 4, 128).transpose(3, 0, 4, 1, 2, 5)
        .astype(f8))
    vq = np.stack([v1, v2]).astype(np.float32)
    v2a = np.ascontiguousarray(np.broadcast_to(
        vq.reshape(2, 2, 2, 128).transpose(3, 0, 1, 2)[..., None],
        (128, 2, 2, 2, 128)).astype(f8))
    bsq = np.stack([b1, b2]).astype(np.float32)
    with_bias = bool(np.any(bsq != 0.0))
    bs = np.ascontiguousarray(bsq.reshape(2, 4, 128).transpose(2, 0, 1))
    # z contribution of an all-zero padded row: exp(v . tanh(b)) per stream
    # (fp8-quantized the same way the device computes it)
    v_q = vq.astype(f8).astype(np.float32)
    b_q = np.tanh(bsq).astype(f8).astype(np.float32)
    pad_z = np.exp((v_q * b_q).sum(axis=1))        # (2,)
    pad_z = np.array([pad_z[0], pad_z[1], pad_z[1]])

    in_maps, pad_counts = [], []
    for core in range(NCORES):
        xt, pc = _build_core_inputs(core, slot_samples, groups, total_cols,
                                    inputs)
        in_maps.append({"xt": xt, "wt2": wt2, "v2": v2a, "bias": bs})
        pad_counts.append(pc)

    cache_key = (total_cols, with_bias) + tuple(
        (g["stream"], g["n"]) for g in groups)
    nc = _PROGRAM_CACHE.get(cache_key)
    if nc is None:
        nc = _build_program(groups, chunks, total_cols, with_bias=with_bias)
        _PROGRAM_CACHE[cache_key] = nc

    results = _run_spmd(nc, in_maps)
    _RUN_STATE["_main_st"] = _RUN_STATE[id(nc)]
    _RUN_STATE["_last_plan"] = (groups, chunks, total_cols, with_bias, in_maps)

    # ---- host finalization (f64, trivially small) ----
    pooled_acc = np.zeros((3, B, H), np.float64)
    z_acc = np.zeros((3, B), np.float64)
    for core in range(NCORES):
        out = results[core]
        pooled = np.asarray(out["pooled"], np.float64)   # (128, 4G)
        zs = np.asarray(out["zs"], np.float64).reshape(G)
        for gi, g in enumerate(groups):
            samp = slot_samples[(g["stream"], g["j"])][core]
            vec = pooled[:, 4 * gi:4 * gi + 4].T.reshape(H)
            pooled_acc[g["stream"], samp] += vec
            z_acc[g["stream"], samp] += zs[gi]
        z_acc -= pad_counts[core] * pad_z[:, None]

    reprs = np.tanh(pooled_acc / z_acc[:, :, None])
    code, anc, neg = reprs

    def cos(x, y):
        num = (x * y).sum(axis=1)
        den = np.linalg.norm(x, axis=1) * np.linalg.norm(y, axis=1) + EPS
        return num / den

    loss = np.mean(np.clip(MARGIN - cos(code, anc) + cos(code, neg), 1e-6, None))
    return np.float32(loss)
